# revision 14
# baseline (speedup 1.0000x reference)
"""Trainium2 8-core kernel for the Contrast module (fp8 DoubleRow + split AG).

    za_p = ELU(za @ W1 + b1) @ W2 + b2          (same for zb)
    za_ca = softmax((za_p Wq + bq)(zb_p Wk + bk)^T / sqrt(256)) @ (zb_p Wv + bv)
    zb_ca = softmax((zb_p Wq + bq)(za_p Wk + bk)^T / sqrt(256)) @ (za_p Wv + bv)
    out = concat(za_ca, zb_ca, axis=1)

Key structural choices:
 - Wk is folded into the query side on the host: with Wqk = Wq Wk^T / s and
   bqk = bq Wk^T / s, softmax(Q K^T/s) == softmax((P_q Wqk + bqk) P_k^T)
   (the dropped (P_q Wq + bq)·bk^T term is constant along k, so it cancels
   in softmax). This removes the entire K projection: the score matmul
   contracts Q'' (fp8) against the raw projections P_k (fp8).
 - fp8 scaling: Wqk carries 1/4 and the P_k fp8 copy carries 1/4 (product
   1/16 = 1/sqrt(D/2)); splitting the scale keeps both operands out of the
   fp8e4 subnormal range.
 - TWO AllGathers, one per attention direction, each fully overlapped:
   AG(dir b) ships P_a/V_a during the zb projection; AG(dir a) ships
   P_b/V_b during direction-b attention. Neither exposes collective time
   on the PE critical path (the single-AG variant stalled PE ~80us).
 - Attention matmuls (scores and attn@V) run in fp8 DoubleRow perf mode;
   PSUM accumulation stays f32.
 - Projections (W1/ELU/W2, V, Q'') stay f32r for accuracy.

Sharding: rows data-parallel across 8 cores; weights replicated.
Layout notes:
 - Activations flow feature-major ("transposed"): zaT [h, n] so every
   matmul contracts over the partition axis without on-chip transposes.
 - softmax denominator: V panels carry a ones column; attn@V is split into
   N=256 and N=258 matmuls so the rowsum accumulates in PSUM col 512.
 - No max-subtraction in softmax: scores are ~N(0, 0.85^2); exp(s-2) keeps
   fp8 exp outputs in range, and the e^-2 cancels against the denominator.
 - ELU+1 = max(x+1, min(exp(x), 1)) via one Act + two DVE ops
   (scalar_tensor_tensor fuses the min+max); the -1 is folded into b2.
 - bv is added at finalize (softmax weights sum to 1).
"""

import numpy as np

import concourse.mybir as mybir
import concourse.tile as tile
from concourse import bacc
from concourse.bass_utils import run_bass_kernel_spmd

dt = mybir.dt
AF = mybir.ActivationFunctionType
ALU = mybir.AluOpType
PM = mybir.MatmulPerfMode

R = 8            # cores
N = 8192         # total rows
H = 1024         # hidden
D = 512          # attention dim
NL = N // R      # rows per core
HC = H // 128    # 8 h-chunks
DC = D // 128    # 4 d-chunks
NB = NL // 512   # 2 n-blocks per core slice
KC = NL // 128   # 8 key-chunks per shard
SCALE = 16.0     # sqrt(512/2)
QS = 4.0         # per-operand fp8 scale split: (Q''/4)·(P/4) = Q''P/16
KVF = D * NL     # elements per P (or V) shard section
SHARD = 2 * KVF  # P_x | V_x, fp8 bytes per core per direction

F32R = dt.float32r
BF16 = dt.bfloat16
F8 = dt.float8e4


def _r(ap):
    return ap.bitcast(F32R)


def build():
    nc = bacc.Bacc("TRN2", target_bir_lowering=False, debug=False, num_devices=R)

    def inp(name, shape, dtype=dt.float32):
        return nc.dram_tensor(name, shape, dtype, kind="ExternalInput")

    zT = {
        "a": inp("zaT", [128, HC, NL], BF16),
        "b": inp("zbT", [128, HC, NL], BF16),
    }
    w1 = inp("W1t", [128, HC, H], BF16)
    w2 = inp("W2t", [128, HC, D])
    wqk = inp("Wqkt", [128, DC, D], BF16)
    wv = inp("Wvt", [128, DC, D], BF16)
    b1d = inp("b1t", [128, HC])
    b1p1d = inp("b1p1t", [128, HC])
    b2d = inp("b2t", [128, DC])
    bqkd = inp("bqkt", [128, DC])
    bvd = inp("bvt", [128, D])
    vpadd = inp("vpad", [128, 2 * KC], F8)
    out_d = nc.dram_tensor("out", [NL, 2 * D], dt.float32, kind="ExternalOutput")

    with tile.TileContext(nc) as tc:
        psum = tc.alloc_tile_pool(name="psum", bufs=1, space="PSUM")
        dram = tc.alloc_tile_pool(name="dram", bufs=1, space="DRAM")
        const = tc.alloc_tile_pool(name="const", bufs=1)
        qtp = tc.alloc_tile_pool(name="qtp", bufs=1)
        wkvp = tc.alloc_tile_pool(name="wkvp", bufs=1)
        projp = tc.alloc_tile_pool(name="projp", bufs=1)

        # ---- DMAs ordered by first PE use: (w1t[hc], z[hc]) pairs so the
        # hc-accumulation of the first W1 block starts as soon as each pair
        # lands; biases next (first ELU needs them only ~2us in).
        w1t = projp.tile([128, HC, H], BF16, name="w1t")
        zt = {}
        b1 = const.tile([128, HC], dt.float32, name="b1")
        b1p1 = const.tile([128, HC], dt.float32, name="b1p1")
        b2 = const.tile([128, DC], dt.float32, name="b2")
        bqk = const.tile([128, DC], dt.float32, name="bqk")
        bv = const.tile([128, D], dt.float32, name="bv")
        z0 = projp.tile([128, HC, 512], BF16, tag="z", bufs=2, name="z_a0")
        zt[("a", 0)] = z0
        for hc in range(HC):
            nc.sync.dma_start(w1t[:, hc, :], w1.ap()[:, hc, :])
            nc.gpsimd.dma_start(z0[:, hc, :], zT["a"].ap()[:, hc, 0:512])
        nc.sync.dma_start(b1[:], b1d.ap())
        nc.sync.dma_start(b1p1[:], b1p1d.ap())
        z1 = projp.tile([128, HC, 512], BF16, tag="z", bufs=2, name="z_a1")
        zt[("a", 1)] = z1
        for hc in range(HC):
            nc.gpsimd.dma_start(z1[:, hc, :], zT["a"].ap()[:, hc, 512:1024])
        w2t = projp.tile([128, HC, D], F32R, name="w2t")
        nc.sync.dma_start(b2[:], b2d.ap())
        nc.sync.dma_start(w2t[:], _r(w2.ap()))
        wqkt = wkvp.tile([128, DC, D], BF16, name="wqkt")
        wvt = wkvp.tile([128, DC, D], BF16, name="wvt")
        nc.sync.dma_start(wvt[:], wv.ap())
        nc.sync.dma_start(wqkt[:], wqk.ap())
        nc.sync.dma_start(bqk[:], bqkd.ap())
        nc.sync.dma_start(bv[:], bvd.ap())
        # exp(x - 2): keeps fp8 exp outputs in range for scores up to ~7.4;
        # the e^-2 cancels between numerator and the ones-column denominator.
        negc = const.tile([128, 1], dt.float32, name="negc")
        nc.vector.memset(negc[:], -2.0)

        pT = {
            "a": wkvp.tile([128, DC, NL], BF16, name="pta"),
            "b": wkvp.tile([128, DC, NL], BF16, name="ptb"),
        }

        # per-direction fused AG buffers: [P_x | V_x] fp8
        agin = {x: dram.tile([SHARD], F8, name=f"agin_{x}") for x in ("b", "a")}
        agout = {
            x: dram.tile([R * SHARD], F8, name=f"agout_{x}", addr_space="Shared")
            for x in ("b", "a")
        }

        # ================= projection + P/V shards =================
        def mmpair(g):
            """psum half-slot g within double-width [128, 2, 512] tiles."""
            if g % 2 == 0:
                mmpair.cur = psum.tile(
                    [128, 2, 512], dt.float32, tag="mm", bufs=2, name="ps2"
                )
            return mmpair.cur[:, g % 2, :]

        for src, other in (("a", "b"), ("b", "a")):
            for nb in range(NB):
                ns = slice(nb * 512, (nb + 1) * 512)
                if (src, nb) in zt:
                    z = zt[(src, nb)]
                else:
                    z = projp.tile(
                        [128, HC, 512], BF16, tag="z", bufs=2, name=f"z_{src}{nb}"
                    )
                    for hc in range(HC):
                        nc.gpsimd.dma_start(z[:, hc, :], zT[src].ap()[:, hc, ns])
                hT = projp.tile([128, HC, 512], F32R, tag="h", bufs=2, name=f"h_{src}{nb}")
                # ELU(x)+1 = max(x+1, min(exp(x), 1)), x = ps + b1.
                # The combine (stt) is software-pipelined one block behind so
                # the DVE's PSUM reader (xp1) is never queued behind a combine
                # and the PSUM slot frees right after Act's Exp + DVE's add.
                stt_q = []
                for d1c in range(HC):
                    ps = mmpair(d1c)
                    for hc in range(HC):
                        nc.tensor.matmul(
                            ps,
                            w1t[:, hc, d1c * 128 : (d1c + 1) * 128],
                            z[:, hc, :],
                            start=(hc == 0),
                            stop=(hc == HC - 1),
                        )
                    e = projp.tile([128, 512], dt.float32, tag="e", bufs=3, name="e")
                    xp1 = projp.tile([128, 512], dt.float32, tag="xp1", bufs=3, name="xp1")
                    nc.scalar.activation(e[:], ps, AF.Exp, bias=b1[:, d1c : d1c + 1])
                    nc.vector.tensor_scalar(
                        xp1[:], ps, b1p1[:, d1c : d1c + 1], None, ALU.add
                    )
                    stt_q.append((d1c, e, xp1))
                    if len(stt_q) > 1:
                        dd, ee, xx = stt_q.pop(0)
                        nc.vector.scalar_tensor_tensor(
                            hT[:, dd, :], ee[:], 1.0, xx[:], ALU.min, ALU.max
                        )
                dd, ee, xx = stt_q.pop(0)
                nc.vector.scalar_tensor_tensor(
                    hT[:, dd, :], ee[:], 1.0, xx[:], ALU.min, ALU.max
                )
                for d2c in range(DC):
                    ps = mmpair(d2c)
                    for d1c in range(HC):
                        nc.tensor.matmul(
                            ps,
                            w2t[:, d1c, d2c * 128 : (d2c + 1) * 128],
                            hT[:, d1c, :],
                            start=(d1c == 0),
                            stop=(d1c == HC - 1),
                        )
                    nc.scalar.activation(
                        pT[src][:, d2c, ns], ps, AF.Identity, bias=b2[:, d2c : d2c + 1]
                    )

            # stage this src's P (fp8, /4) + V into the *other* direction's
            # AG shard, then fire that direction's AllGather.
            pv = agin[other][0:KVF].rearrange("(d n) -> d n", n=NL)
            vv = agin[other][KVF : 2 * KVF].rearrange("(n d) -> n d", d=D)
            for dc in range(DC):
                for nb in range(NB):
                    ns = slice(nb * 512, (nb + 1) * 512)
                    s = projp.tile([128, 512], F8, tag="stg", bufs=4, name="stg_p")
                    nc.scalar.activation(
                        s[:], pT[src][:, dc, ns], AF.Copy, scale=1.0 / QS
                    )
                    nc.gpsimd.dma_start(pv[dc * 128 : (dc + 1) * 128, ns], s[:])
            for nt in range(KC):
                ps = mmpair(nt)
                for d2c in range(DC):
                    nc.tensor.matmul(
                        ps,
                        pT[src][:, d2c, nt * 128 : (nt + 1) * 128],
                        wvt[:, d2c, :],
                        start=(d2c == 0),
                        stop=(d2c == DC - 1),
                    )
                s = projp.tile([128, 512], F8, tag="stg", bufs=4, name="stg_v")
                nc.scalar.activation(s[:], ps, AF.Copy)
                nc.gpsimd.dma_start(vv[nt * 128 : (nt + 1) * 128, :], s[:])
            nc.gpsimd.collective_compute(
                "AllGather",
                ALU.bypass,
                ins=[agin[other].opt()],
                outs=[agout[other].opt()],
                replica_groups=[list(range(R))],
            )

        projp.release()

        # ================= queries Q'' (overlap the 2nd AG) =================
        qT = {}
        for x in ("b", "a"):
            qT[x] = qtp.tile([128, DC, NL], F8, name=f"qt_{x}")
            for g, (dc, nb) in enumerate(
                (dc, nb) for dc in range(DC) for nb in range(NB)
            ):
                ps = mmpair(g)
                for d2c in range(DC):
                    nc.tensor.matmul(
                        ps,
                        wqkt[:, d2c, dc * 128 : (dc + 1) * 128],
                        pT[x][:, d2c, nb * 512 : (nb + 1) * 512],
                        start=(d2c == 0),
                        stop=(d2c == DC - 1),
                    )
                nc.scalar.activation(
                    qT[x][:, dc, nb * 512 : (nb + 1) * 512],
                    ps,
                    AF.Identity,
                    bias=bqk[:, dc : dc + 1],
                )
        wkvp.release()

        # ================= attention (fp8 DoubleRow) =================
        # Software-pipelined: block i+1's score matmuls are issued before
        # block i's attn@V so the Exp activations never wait on PE.
        attnp = tc.alloc_tile_pool(name="attnp", bufs=1)

        accs = {}
        pending = []  # deferred attn@V closures, one per (x, r, qb) block

        def do_scores(x, r, qb, ktile):
            qs = slice(qb * 512, (qb + 1) * 512)
            exps = []
            for kp in range(KC // 2):
                ex = attnp.tile([128, 2, 512], F8, tag="exp", bufs=16, name="ex")
                ps2 = psum.tile(
                    [128, 2, 512], dt.float32, tag="mm", bufs=2, name="ps_s"
                )
                for j in range(2):
                    kt_i = 2 * kp + j
                    for c in range(DC // 2):
                        nc.tensor.matmul(
                            ps2[:, j, :],
                            ktile[:, 2 * c : 2 * c + 2,
                                  kt_i * 128 : (kt_i + 1) * 128],
                            qT[x][:, 2 * c : 2 * c + 2, qs],
                            start=(c == 0),
                            stop=(c == DC // 2 - 1),
                            perf_mode=PM.DoubleRow,
                        )
                nc.scalar.activation(ex[:], ps2[:], AF.Exp, bias=negc[:])
                exps.append(ex)
            return exps

        def do_attnv(x, col, r, qb, exps, vtile):
            for qt_i in range(4):
                qsl = slice(qt_i * 128, (qt_i + 1) * 128)
                p1 = psum.tile([128, 256], dt.float32, tag="po1", bufs=2, name="po1")
                p2 = psum.tile([128, 258], dt.float32, tag="po2", bufs=2, name="po2")
                for kp in range(KC // 2):
                    nc.tensor.matmul(
                        p1[:],
                        exps[kp][:, :, qsl],
                        vtile[:, 2 * kp : 2 * kp + 2, 0:256],
                        start=(kp == 0),
                        stop=(kp == KC // 2 - 1),
                        perf_mode=PM.DoubleRow,
                    )
                    nc.tensor.matmul(
                        p2[:],
                        exps[kp][:, :, qsl],
                        vtile[:, 2 * kp : 2 * kp + 2, 256 : D + 2],
                        start=(kp == 0),
                        stop=(kp == KC // 2 - 1),
                        perf_mode=PM.DoubleRow,
                    )
                if r == 0:
                    acc = attnp.tile(
                        [128, D + 2], dt.float32, tag="acc", bufs=8,
                        name=f"acc{qb}{qt_i}",
                    )
                    accs[(x, qb, qt_i)] = acc
                    nc.vector.tensor_copy(acc[:, 0:256], p1[:])
                    nc.vector.tensor_copy(acc[:, 256 : D + 2], p2[:])
                else:
                    acc = accs[(x, qb, qt_i)]
                    nc.vector.tensor_tensor(
                        acc[:, 0:256], acc[:, 0:256], p1[:], ALU.add
                    )
                    nc.vector.tensor_tensor(
                        acc[:, 256 : D + 2], acc[:, 256 : D + 2], p2[:], ALU.add
                    )
                if r == R - 1:
                    # finalize: out = acc[:, :512] / acc[:, 512] + bv
                    rr = attnp.tile([128, 1], dt.float32, tag="rr", bufs=4, name="rr")
                    nc.vector.reciprocal(rr[:], acc[:, D : D + 1])
                    ot = attnp.tile([128, D], dt.float32, tag="ot", bufs=3, name="ot")
                    nc.vector.scalar_tensor_tensor(
                        ot[:], acc[:, 0:D], rr[:], bv[:], ALU.mult, ALU.add
                    )
                    r0 = qb * 512 + qt_i * 128
                    nc.gpsimd.dma_start(
                        out_d.ap()[r0 : r0 + 128, col * D : (col + 1) * D], ot[:]
                    )

        for x, col in (("b", 1), ("a", 0)):
            for r in range(R):
                base = r * SHARD
                ktile = attnp.tile([128, DC, NL], F8, tag="kt", bufs=2, name=f"kt{r}")
                nc.scalar.dma_start(
                    ktile[:],
                    agout[x][base : base + KVF].rearrange(
                        "(dc p n) -> p dc n", p=128, n=NL
                    ),
                )
                vtile = attnp.tile(
                    [128, KC, D + 2], F8, tag="vt", bufs=3, name=f"vt{r}"
                )
                nc.scalar.dma_start(
                    vtile[:, :, 0:D],
                    agout[x][base + KVF : base + 2 * KVF].rearrange(
                        "(kc p d) -> p kc d", p=128, d=D
                    ),
                )
                nc.scalar.dma_start(
                    vtile[:, :, D : D + 2],
                    vpadd.ap().rearrange("p (kc c) -> p kc c", c=2),
                )
                for qb in range(NB):
                    exps = do_scores(x, r, qb, ktile)
                    pending.append((x, col, r, qb, exps, vtile))
                    if len(pending) > 1:
                        do_attnv(*pending.pop(0))
        while pending:
            do_attnv(*pending.pop(0))
        attnp.release()
        qtp.release()
        const.release()
        dram.release()
        psum.release()

    nc.compile()
    return nc


_NC = None


def _get_nc():
    global _NC
    if _NC is None:
        _NC = build()
    return _NC


def _chunk_w(w):
    """[X, Y] -> [128, X//128, Y] partition-chunked, contiguous."""
    x, y = w.shape
    return np.ascontiguousarray(w.reshape(x // 128, 128, y).transpose(1, 0, 2))


def _chunk_b(b):
    return np.ascontiguousarray(np.asarray(b, np.float32).reshape(-1, 128).T)


def prep_in_maps(za, zb, W1, b1, W2, b2, Wq, bq, Wk, bk, Wv, bv):
    za = np.asarray(za, np.float32)
    zb = np.asarray(zb, np.float32)
    W1 = np.asarray(W1, np.float32)
    W2 = np.asarray(W2, np.float32)
    Wq = np.asarray(Wq, np.float32)
    Wk = np.asarray(Wk, np.float32)
    Wv = np.asarray(Wv, np.float32)
    b1 = np.asarray(b1, np.float32)
    b2 = np.asarray(b2, np.float32)
    bq = np.asarray(bq, np.float32)
    bk = np.asarray(bk, np.float32)
    bv = np.asarray(bv, np.float32)

    f8 = dt.np(F8)
    bf = dt.np(BF16)
    # Wk folded into the query side; 1/SCALE split as 1/QS per fp8 operand.
    Wqk = (Wq @ Wk.T) * (QS / SCALE)
    bqk = (bq @ Wk.T) * (QS / SCALE)
    shared = {
        "W1t": _chunk_w(W1).astype(bf),
        "W2t": _chunk_w(W2),
        "Wqkt": _chunk_w(Wqk).astype(bf),
        "Wvt": _chunk_w(Wv).astype(bf),
        "b1t": _chunk_b(b1),
        "b1p1t": _chunk_b(b1 + 1.0),
        "b2t": _chunk_b(b2 - W2.sum(axis=0)),
        "bqkt": _chunk_b(bqk),
        "bvt": np.ascontiguousarray(np.broadcast_to(bv, (128, D)).astype(np.float32)),
        "vpad": np.ascontiguousarray(
            np.broadcast_to(
                np.tile(np.array([1.0, 0.0], np.float32), KC), (128, 2 * KC)
            )
        ).astype(f8),
    }
    zaT = np.ascontiguousarray(za.T).astype(bf)  # [H, N]
    zbT = np.ascontiguousarray(zb.T).astype(bf)
    in_maps = []
    for c in range(R):
        cs = slice(c * NL, (c + 1) * NL)
        in_maps.append(
            {
                "zaT": _chunk_w(zaT[:, cs]),
                "zbT": _chunk_w(zbT[:, cs]),
                **shared,
            }
        )
    return in_maps


def kernel(**inputs) -> np.ndarray:
    nc = _get_nc()
    in_maps = prep_in_maps(**inputs)
    res = run_bass_kernel_spmd(nc, in_maps, core_ids=list(range(R)))
    return np.concatenate([res.results[c]["out"] for c in range(R)], axis=0)


# revision 15
# speedup vs baseline: 1.0226x; 1.0226x over previous
"""Trainium2 8-core kernel for the Contrast module (fp8 DoubleRow + split AG).

    za_p = ELU(za @ W1 + b1) @ W2 + b2          (same for zb)
    za_ca = softmax((za_p Wq + bq)(zb_p Wk + bk)^T / sqrt(256)) @ (zb_p Wv + bv)
    zb_ca = softmax((zb_p Wq + bq)(za_p Wk + bk)^T / sqrt(256)) @ (za_p Wv + bv)
    out = concat(za_ca, zb_ca, axis=1)

Key structural choices:
 - Wk is folded into the query side on the host: with Wqk = Wq Wk^T / s and
   bqk = bq Wk^T / s, softmax(Q K^T/s) == softmax((P_q Wqk + bqk) P_k^T)
   (the dropped (P_q Wq + bq)·bk^T term is constant along k, so it cancels
   in softmax). This removes the entire K projection: the score matmul
   contracts Q'' (fp8) against the raw projections P_k (fp8).
 - fp8 scaling: Wqk carries 1/4 and the P_k fp8 copy carries 1/4 (product
   1/16 = 1/sqrt(D/2)); splitting the scale keeps both operands out of the
   fp8e4 subnormal range.
 - TWO AllGathers, one per attention direction, each fully overlapped:
   AG(dir b) ships P_a/V_a during the zb projection; AG(dir a) ships
   P_b/V_b during direction-b attention. Neither exposes collective time
   on the PE critical path (the single-AG variant stalled PE ~80us).
 - Attention matmuls (scores and attn@V) run in fp8 DoubleRow perf mode;
   PSUM accumulation stays f32.
 - Projections (W1/ELU/W2, V, Q'') stay f32r for accuracy.

Sharding: rows data-parallel across 8 cores; weights replicated.
Layout notes:
 - Activations flow feature-major ("transposed"): zaT [h, n] so every
   matmul contracts over the partition axis without on-chip transposes.
 - softmax denominator: V panels carry a ones column; attn@V is split into
   N=256 and N=258 matmuls so the rowsum accumulates in PSUM col 512.
 - No max-subtraction in softmax: scores are ~N(0, 0.85^2); exp(s-2) keeps
   fp8 exp outputs in range, and the e^-2 cancels against the denominator.
 - ELU+1 = max(x+1, min(exp(x), 1)) via one Act + two DVE ops
   (scalar_tensor_tensor fuses the min+max); the -1 is folded into b2.
 - bv is added at finalize (softmax weights sum to 1).
"""

import numpy as np

import concourse.mybir as mybir
import concourse.tile as tile
from concourse import bacc
from concourse.bass_utils import run_bass_kernel_spmd

dt = mybir.dt
AF = mybir.ActivationFunctionType
ALU = mybir.AluOpType
PM = mybir.MatmulPerfMode

R = 8            # cores
N = 8192         # total rows
H = 1024         # hidden
D = 512          # attention dim
NL = N // R      # rows per core
HC = H // 128    # 8 h-chunks
DC = D // 128    # 4 d-chunks
NB = NL // 512   # 2 n-blocks per core slice
KC = NL // 128   # 8 key-chunks per shard
SCALE = 16.0     # sqrt(512/2)
QS = 4.0         # per-operand fp8 scale split: (Q''/4)·(P/4) = Q''P/16
KVF = D * NL     # elements per P (or V) shard section
SHARD = 2 * KVF  # P_x | V_x, fp8 bytes per core per direction

F32R = dt.float32r
BF16 = dt.bfloat16
F8 = dt.float8e4


def _r(ap):
    return ap.bitcast(F32R)


def build():
    nc = bacc.Bacc("TRN2", target_bir_lowering=False, debug=False, num_devices=R)

    def inp(name, shape, dtype=dt.float32):
        return nc.dram_tensor(name, shape, dtype, kind="ExternalInput")

    zT = {
        "a": inp("zaT", [128, HC, NL], BF16),
        "b": inp("zbT", [128, HC, NL], BF16),
    }
    w1 = inp("W1t", [128, HC, H], BF16)
    w2 = inp("W2t", [128, HC, D])
    wqk = inp("Wqkt", [128, DC, D], BF16)
    wv = inp("Wvt", [128, DC, D], BF16)
    b1d = inp("b1t", [128, HC])
    b1p1d = inp("b1p1t", [128, HC])
    b2d = inp("b2t", [128, DC])
    bqkd = inp("bqkt", [128, DC])
    bvd = inp("bvt", [128, D])
    vpadd = inp("vpad", [128, 2 * KC], F8)
    out_d = nc.dram_tensor("out", [NL, 2 * D], dt.float32, kind="ExternalOutput")

    with tile.TileContext(nc) as tc:
        psum = tc.alloc_tile_pool(name="psum", bufs=1, space="PSUM")
        dram = tc.alloc_tile_pool(name="dram", bufs=1, space="DRAM")
        const = tc.alloc_tile_pool(name="const", bufs=1)
        qtp = tc.alloc_tile_pool(name="qtp", bufs=1)
        wkvp = tc.alloc_tile_pool(name="wkvp", bufs=1)
        projp = tc.alloc_tile_pool(name="projp", bufs=1)

        # ---- DMAs ordered by first PE use: (w1t[hc], z[hc]) pairs so the
        # hc-accumulation of the first W1 block starts as soon as each pair
        # lands; biases next (first ELU needs them only ~2us in).
        w1t = projp.tile([128, HC, H], BF16, name="w1t")
        zt = {}
        b1 = const.tile([128, HC], dt.float32, name="b1")
        b1p1 = const.tile([128, HC], dt.float32, name="b1p1")
        b2 = const.tile([128, DC], dt.float32, name="b2")
        bqk = const.tile([128, DC], dt.float32, name="bqk")
        bv = const.tile([128, D], dt.float32, name="bv")
        z0 = projp.tile([128, HC, 512], BF16, tag="z", bufs=2, name="z_a0")
        zt[("a", 0)] = z0
        for hc in range(HC):
            nc.sync.dma_start(w1t[:, hc, :], w1.ap()[:, hc, :])
            nc.scalar.dma_start(z0[:, hc, :], zT["a"].ap()[:, hc, 0:512])
        nc.sync.dma_start(b1[:], b1d.ap())
        nc.sync.dma_start(b1p1[:], b1p1d.ap())
        z1 = projp.tile([128, HC, 512], BF16, tag="z", bufs=2, name="z_a1")
        zt[("a", 1)] = z1
        for hc in range(HC):
            nc.scalar.dma_start(z1[:, hc, :], zT["a"].ap()[:, hc, 512:1024])
        w2t = projp.tile([128, HC, D], F32R, name="w2t")
        nc.sync.dma_start(b2[:], b2d.ap())
        nc.sync.dma_start(w2t[:], _r(w2.ap()))
        wqkt = wkvp.tile([128, DC, D], BF16, name="wqkt")
        wvt = wkvp.tile([128, DC, D], BF16, name="wvt")
        nc.sync.dma_start(wvt[:], wv.ap())
        nc.sync.dma_start(wqkt[:], wqk.ap())
        nc.sync.dma_start(bqk[:], bqkd.ap())
        nc.sync.dma_start(bv[:], bvd.ap())
        # exp(x - 2): keeps fp8 exp outputs in range for scores up to ~7.4;
        # the e^-2 cancels between numerator and the ones-column denominator.
        negc = const.tile([128, 1], dt.float32, name="negc")
        nc.vector.memset(negc[:], -2.0)

        pT = {
            "a": wkvp.tile([128, DC, NL], BF16, name="pta"),
            "b": wkvp.tile([128, DC, NL], BF16, name="ptb"),
        }

        # per-direction fused AG buffers: [P_x | V_x] fp8
        agin = {x: dram.tile([SHARD], F8, name=f"agin_{x}") for x in ("b", "a")}
        agout = {
            x: dram.tile([R * SHARD], F8, name=f"agout_{x}", addr_space="Shared")
            for x in ("b", "a")
        }

        # ================= projection + P/V shards =================
        def mmpair(g):
            """psum half-slot g within double-width [128, 2, 512] tiles."""
            if g % 2 == 0:
                mmpair.cur = psum.tile(
                    [128, 2, 512], dt.float32, tag="mm", bufs=2, name="ps2"
                )
            return mmpair.cur[:, g % 2, :]

        for src, other in (("a", "b"), ("b", "a")):
            for nb in range(NB):
                ns = slice(nb * 512, (nb + 1) * 512)
                if (src, nb) in zt:
                    z = zt[(src, nb)]
                else:
                    z = projp.tile(
                        [128, HC, 512], BF16, tag="z", bufs=2, name=f"z_{src}{nb}"
                    )
                    for hc in range(HC):
                        nc.scalar.dma_start(z[:, hc, :], zT[src].ap()[:, hc, ns])
                hT = projp.tile([128, HC, 512], F32R, tag="h", bufs=2, name=f"h_{src}{nb}")
                # ELU(x)+1 = max(x+1, min(exp(x), 1)), x = ps + b1.
                # The combine (stt) is software-pipelined one block behind so
                # the DVE's PSUM reader (xp1) is never queued behind a combine
                # and the PSUM slot frees right after Act's Exp + DVE's add.
                stt_q = []
                for d1c in range(HC):
                    ps = mmpair(d1c)
                    for hc in range(HC):
                        nc.tensor.matmul(
                            ps,
                            w1t[:, hc, d1c * 128 : (d1c + 1) * 128],
                            z[:, hc, :],
                            start=(hc == 0),
                            stop=(hc == HC - 1),
                        )
                    e = projp.tile([128, 512], dt.float32, tag="e", bufs=3, name="e")
                    xp1 = projp.tile([128, 512], dt.float32, tag="xp1", bufs=3, name="xp1")
                    nc.scalar.activation(e[:], ps, AF.Exp, bias=b1[:, d1c : d1c + 1])
                    nc.vector.tensor_scalar(
                        xp1[:], ps, b1p1[:, d1c : d1c + 1], None, ALU.add
                    )
                    stt_q.append((d1c, e, xp1))
                    if len(stt_q) > 1:
                        dd, ee, xx = stt_q.pop(0)
                        nc.vector.scalar_tensor_tensor(
                            hT[:, dd, :], ee[:], 1.0, xx[:], ALU.min, ALU.max
                        )
                dd, ee, xx = stt_q.pop(0)
                nc.vector.scalar_tensor_tensor(
                    hT[:, dd, :], ee[:], 1.0, xx[:], ALU.min, ALU.max
                )
                for d2c in range(DC):
                    ps = mmpair(d2c)
                    for d1c in range(HC):
                        nc.tensor.matmul(
                            ps,
                            w2t[:, d1c, d2c * 128 : (d2c + 1) * 128],
                            hT[:, d1c, :],
                            start=(d1c == 0),
                            stop=(d1c == HC - 1),
                        )
                    nc.scalar.activation(
                        pT[src][:, d2c, ns], ps, AF.Identity, bias=b2[:, d2c : d2c + 1]
                    )

            # stage this src's P (fp8, /4) + V into the *other* direction's
            # AG shard, then fire that direction's AllGather.
            pv = agin[other][0:KVF].rearrange("(d n) -> d n", n=NL)
            vv = agin[other][KVF : 2 * KVF].rearrange("(n d) -> n d", d=D)
            for dc in range(DC):
                for nb in range(NB):
                    ns = slice(nb * 512, (nb + 1) * 512)
                    s = projp.tile([128, 512], F8, tag="stg", bufs=4, name="stg_p")
                    nc.scalar.activation(
                        s[:], pT[src][:, dc, ns], AF.Copy, scale=1.0 / QS
                    )
                    nc.sync.dma_start(pv[dc * 128 : (dc + 1) * 128, ns], s[:])
            for nt in range(KC):
                ps = mmpair(nt)
                for d2c in range(DC):
                    nc.tensor.matmul(
                        ps,
                        pT[src][:, d2c, nt * 128 : (nt + 1) * 128],
                        wvt[:, d2c, :],
                        start=(d2c == 0),
                        stop=(d2c == DC - 1),
                    )
                s = projp.tile([128, 512], F8, tag="stg", bufs=4, name="stg_v")
                nc.scalar.activation(s[:], ps, AF.Copy)
                nc.sync.dma_start(vv[nt * 128 : (nt + 1) * 128, :], s[:])
            nc.gpsimd.collective_compute(
                "AllGather",
                ALU.bypass,
                ins=[agin[other].opt()],
                outs=[agout[other].opt()],
                replica_groups=[list(range(R))],
            )

        projp.release()

        # ================= queries Q'' (overlap the 2nd AG) =================
        qT = {}
        for x in ("b", "a"):
            qT[x] = qtp.tile([128, DC, NL], F8, name=f"qt_{x}")
            for g, (dc, nb) in enumerate(
                (dc, nb) for dc in range(DC) for nb in range(NB)
            ):
                ps = mmpair(g)
                for d2c in range(DC):
                    nc.tensor.matmul(
                        ps,
                        wqkt[:, d2c, dc * 128 : (dc + 1) * 128],
                        pT[x][:, d2c, nb * 512 : (nb + 1) * 512],
                        start=(d2c == 0),
                        stop=(d2c == DC - 1),
                    )
                nc.scalar.activation(
                    qT[x][:, dc, nb * 512 : (nb + 1) * 512],
                    ps,
                    AF.Identity,
                    bias=bqk[:, dc : dc + 1],
                )
        wkvp.release()

        # ================= attention (fp8 DoubleRow) =================
        # Software-pipelined: block i+1's score matmuls are issued before
        # block i's attn@V so the Exp activations never wait on PE.
        attnp = tc.alloc_tile_pool(name="attnp", bufs=1)

        accs = {}
        pending = []  # deferred attn@V closures, one per (x, r, qb) block

        def do_scores(x, r, qb, ktile):
            qs = slice(qb * 512, (qb + 1) * 512)
            exps = []
            for kp in range(KC // 2):
                ex = attnp.tile([128, 2, 512], F8, tag="exp", bufs=16, name="ex")
                ps2 = psum.tile(
                    [128, 2, 512], dt.float32, tag="mm", bufs=2, name="ps_s"
                )
                for j in range(2):
                    kt_i = 2 * kp + j
                    for c in range(DC // 2):
                        nc.tensor.matmul(
                            ps2[:, j, :],
                            ktile[:, 2 * c : 2 * c + 2,
                                  kt_i * 128 : (kt_i + 1) * 128],
                            qT[x][:, 2 * c : 2 * c + 2, qs],
                            start=(c == 0),
                            stop=(c == DC // 2 - 1),
                            perf_mode=PM.DoubleRow,
                        )
                nc.scalar.activation(ex[:], ps2[:], AF.Exp, bias=negc[:])
                exps.append(ex)
            return exps

        def do_attnv(x, col, r, qb, exps, vtile):
            for qt_i in range(4):
                qsl = slice(qt_i * 128, (qt_i + 1) * 128)
                p1 = psum.tile([128, 256], dt.float32, tag="po1", bufs=2, name="po1")
                p2 = psum.tile([128, 258], dt.float32, tag="po2", bufs=2, name="po2")
                for kp in range(KC // 2):
                    nc.tensor.matmul(
                        p1[:],
                        exps[kp][:, :, qsl],
                        vtile[:, 2 * kp : 2 * kp + 2, 0:256],
                        start=(kp == 0),
                        stop=(kp == KC // 2 - 1),
                        perf_mode=PM.DoubleRow,
                    )
                    nc.tensor.matmul(
                        p2[:],
                        exps[kp][:, :, qsl],
                        vtile[:, 2 * kp : 2 * kp + 2, 256 : D + 2],
                        start=(kp == 0),
                        stop=(kp == KC // 2 - 1),
                        perf_mode=PM.DoubleRow,
                    )
                if r == 0:
                    acc = attnp.tile(
                        [128, D + 2], dt.float32, tag="acc", bufs=8,
                        name=f"acc{qb}{qt_i}",
                    )
                    accs[(x, qb, qt_i)] = acc
                    nc.vector.tensor_copy(acc[:, 0:256], p1[:])
                    nc.vector.tensor_copy(acc[:, 256 : D + 2], p2[:])
                else:
                    acc = accs[(x, qb, qt_i)]
                    nc.vector.tensor_tensor(
                        acc[:, 0:256], acc[:, 0:256], p1[:], ALU.add
                    )
                    nc.vector.tensor_tensor(
                        acc[:, 256 : D + 2], acc[:, 256 : D + 2], p2[:], ALU.add
                    )
                if r == R - 1:
                    # finalize: out = acc[:, :512] / acc[:, 512] + bv
                    rr = attnp.tile([128, 1], dt.float32, tag="rr", bufs=4, name="rr")
                    nc.vector.reciprocal(rr[:], acc[:, D : D + 1])
                    ot = attnp.tile([128, D], dt.float32, tag="ot", bufs=3, name="ot")
                    nc.vector.scalar_tensor_tensor(
                        ot[:], acc[:, 0:D], rr[:], bv[:], ALU.mult, ALU.add
                    )
                    r0 = qb * 512 + qt_i * 128
                    nc.gpsimd.dma_start(
                        out_d.ap()[r0 : r0 + 128, col * D : (col + 1) * D], ot[:]
                    )

        for x, col in (("b", 1), ("a", 0)):
            for r in range(R):
                base = r * SHARD
                ktile = attnp.tile([128, DC, NL], F8, tag="kt", bufs=2, name=f"kt{r}")
                nc.sync.dma_start(
                    ktile[:],
                    agout[x][base : base + KVF].rearrange(
                        "(dc p n) -> p dc n", p=128, n=NL
                    ),
                )
                vtile = attnp.tile(
                    [128, KC, D + 2], F8, tag="vt", bufs=3, name=f"vt{r}"
                )
                nc.sync.dma_start(
                    vtile[:, :, 0:D],
                    agout[x][base + KVF : base + 2 * KVF].rearrange(
                        "(kc p d) -> p kc d", p=128, d=D
                    ),
                )
                nc.sync.dma_start(
                    vtile[:, :, D : D + 2],
                    vpadd.ap().rearrange("p (kc c) -> p kc c", c=2),
                )
                for qb in range(NB):
                    exps = do_scores(x, r, qb, ktile)
                    pending.append((x, col, r, qb, exps, vtile))
                    if len(pending) > 1:
                        do_attnv(*pending.pop(0))
        while pending:
            do_attnv(*pending.pop(0))
        attnp.release()
        qtp.release()
        const.release()
        dram.release()
        psum.release()

    nc.compile()
    return nc


_NC = None


def _get_nc():
    global _NC
    if _NC is None:
        _NC = build()
    return _NC


def _chunk_w(w):
    """[X, Y] -> [128, X//128, Y] partition-chunked, contiguous."""
    x, y = w.shape
    return np.ascontiguousarray(w.reshape(x // 128, 128, y).transpose(1, 0, 2))


def _chunk_b(b):
    return np.ascontiguousarray(np.asarray(b, np.float32).reshape(-1, 128).T)


def prep_in_maps(za, zb, W1, b1, W2, b2, Wq, bq, Wk, bk, Wv, bv):
    za = np.asarray(za, np.float32)
    zb = np.asarray(zb, np.float32)
    W1 = np.asarray(W1, np.float32)
    W2 = np.asarray(W2, np.float32)
    Wq = np.asarray(Wq, np.float32)
    Wk = np.asarray(Wk, np.float32)
    Wv = np.asarray(Wv, np.float32)
    b1 = np.asarray(b1, np.float32)
    b2 = np.asarray(b2, np.float32)
    bq = np.asarray(bq, np.float32)
    bk = np.asarray(bk, np.float32)
    bv = np.asarray(bv, np.float32)

    f8 = dt.np(F8)
    bf = dt.np(BF16)
    # Wk folded into the query side; 1/SCALE split as 1/QS per fp8 operand.
    Wqk = (Wq @ Wk.T) * (QS / SCALE)
    bqk = (bq @ Wk.T) * (QS / SCALE)
    shared = {
        "W1t": _chunk_w(W1).astype(bf),
        "W2t": _chunk_w(W2),
        "Wqkt": _chunk_w(Wqk).astype(bf),
        "Wvt": _chunk_w(Wv).astype(bf),
        "b1t": _chunk_b(b1),
        "b1p1t": _chunk_b(b1 + 1.0),
        "b2t": _chunk_b(b2 - W2.sum(axis=0)),
        "bqkt": _chunk_b(bqk),
        "bvt": np.ascontiguousarray(np.broadcast_to(bv, (128, D)).astype(np.float32)),
        "vpad": np.ascontiguousarray(
            np.broadcast_to(
                np.tile(np.array([1.0, 0.0], np.float32), KC), (128, 2 * KC)
            )
        ).astype(f8),
    }
    zaT = np.ascontiguousarray(za.T).astype(bf)  # [H, N]
    zbT = np.ascontiguousarray(zb.T).astype(bf)
    in_maps = []
    for c in range(R):
        cs = slice(c * NL, (c + 1) * NL)
        in_maps.append(
            {
                "zaT": _chunk_w(zaT[:, cs]),
                "zbT": _chunk_w(zbT[:, cs]),
                **shared,
            }
        )
    return in_maps


def kernel(**inputs) -> np.ndarray:
    nc = _get_nc()
    in_maps = prep_in_maps(**inputs)
    res = run_bass_kernel_spmd(nc, in_maps, core_ids=list(range(R)))
    return np.concatenate([res.results[c]["out"] for c in range(R)], axis=0)
